# revision 12
# baseline (speedup 1.0000x reference)
"""BinaryConv2d (sign-binarized 3x3 conv, stride 1, pad 1) on 8 Trainium2 cores.

Input  x      [32, 128, 56, 56] f32
       weight [256, 128, 3, 3]  f32  (binarized with sign() before the conv)
       b      [256]             f32
Output        [32, 256, 56, 56] f32

Sharding: data-parallel over the batch dim (4 images per core), weights
replicated to all cores.

Device kernel: 1D Winograd F(4,3) along H. Height is tiled into 14 tiles
of 4 output rows; the 6-point input transform v = B^T d runs on HOST
(fp16) and is shipped instead of x. Width stays direct: 3 kw taps
accumulate in PSUM, so PE work is 6/12 of the direct fp16 shift-matmul
conv. Per strip of 7 row-tiles (28 output rows): 18 matmuls (6 t-points x
3 kw) of free 392 into 6 bank-aligned PSUM slots (3 two-bank tiles), plus
one diag(bias) matmul into the t1 slot (t1 has inverse-transform coeff +1
in every output, so bias rides the accumulation for free). ACT evicts the
6 slots to fp16 SBUF in 3 big ops; DVE/GPSIMD form the inverse transform
with fp16 tensor_tensor/tensor_scalar chains (no TensorScalarPtr - it is
~3x slower on DVE) and write the 4 output-row phases as f32 with
contiguous 56-wide inner runs. Measured rel err ~5e-3 (fp16 transforms +
fp16 m eviction, f32 PSUM).
"""

import functools

import numpy as np

P = 128          # partitions == input channels
H = W = 56       # spatial
O = 256          # output channels
NT = 6           # F(4,3) t-points
KW = 3           # kernel cols (direct accumulation)
NI = 14          # height tiles (4 out rows each)
VCOLS = W + 2    # 58 padded input cols
R = 28           # output rows per strip (7 row-tiles)
TI = 7           # row-tiles per strip
NSTRIP = NI // TI  # 2
FREE = TI * W    # 392 matmul free size
N_CORES = 8
N_PER_CORE = 4   # batch 32 / 8 cores

# F(4,3), interpolation points [0, 1, -1, 2, -2, inf]
BT = np.array(
    [
        [4, 0, -5, 0, 1, 0],
        [0, -4, -4, 1, 1, 0],
        [0, 4, -4, -1, 1, 0],
        [0, -2, -1, 2, 1, 0],
        [0, 2, -1, -2, 1, 0],
        [0, 4, 0, -5, 0, 1],
    ],
    np.float64,
)
G = np.array(
    [
        [1 / 4, 0, 0],
        [-1 / 6, -1 / 6, -1 / 6],
        [-1 / 6, 1 / 6, -1 / 6],
        [1 / 24, 1 / 12, 1 / 6],
        [1 / 24, -1 / 12, 1 / 6],
        [0, 0, 1],
    ],
    np.float64,
)
# A^T = [[1,1,1,1,1,0], [0,1,-1,2,-2,0], [0,1,1,4,4,0], [0,1,-1,8,-8,1]]
# with e=m1(+bias), g=m2, h=m3, k=m4, z0=m0, z5=m5 (fp16 in SBUF):
#   Q = e-g, Pp = e+g, S = h-k, Rr = h+k
#   o0 = (z0+Pp)+Rr; o1 = 2S+Q; o2 = 4Rr+Pp; o3 = 8S+(Q+z5)


@functools.lru_cache(maxsize=1)
def _build_nc():
    import concourse.mybir as mybir
    import concourse.tile as tile
    from concourse import bacc

    f16 = mybir.dt.float16
    f32 = mybir.dt.float32

    nc = bacc.Bacc()
    # xp: host-transformed input v[n, c, t, i, col]
    xp = nc.declare_dram_parameter(
        "xp", [N_PER_CORE, P, NT, NI, VCOLS], f16, isOutput=False
    )
    # wt: winograd weights u[c, t, kw, o]
    wt = nc.declare_dram_parameter("wt", [P, NT, KW, O], f16, isOutput=False)
    # bias: diag(b) stationaries per o-half: bias[p, oh, o] = b[oh*128+o]*(p==o)
    bias = nc.declare_dram_parameter("bias", [P, 2, P], f16, isOutput=False)
    out = nc.declare_dram_parameter(
        "out", [N_PER_CORE, O, H, W], f32, isOutput=True
    )
    xp_ap = xp[:]
    wt_ap = wt[:]
    bias_ap = bias[:]
    out_ap = out[:]

    with tile.TileContext(nc) as tc:
        with (
            tc.tile_pool(name="wpool", bufs=1) as wpool,
            tc.tile_pool(name="xpool", bufs=3) as xpool,
            tc.tile_pool(name="spool", bufs=3) as spool,
            tc.tile_pool(name="opool", bufs=4) as opool,
            tc.tile_pool(name="psum", bufs=4, space="PSUM") as pp,
        ):
            # Weights/bias on the scalar (ACT) DMA queue so they don't
            # serialize behind the image loads on sync.
            u_sb = wpool.tile([P, NT, KW, O], f16)
            nc.scalar.dma_start(u_sb[:, 0:3], wt_ap[:, 0:3])
            nc.scalar.dma_start(u_sb[:, 3:6], wt_ap[:, 3:6])
            bd_sb = wpool.tile([P, 2, P], f16)
            nc.scalar.dma_start(bd_sb[:], bias_ap)
            ones_sb = wpool.tile([P, FREE], f16)
            nc.gpsimd.memset(ones_sb[:], 1.0)

            # PE warmup: dummy matmuls with no data deps run during the
            # initial DMA wait and ramp the PE clock before the real stream.
            warm_sb = wpool.tile([P, 448], f16)
            nc.gpsimd.memset(warm_sb[:], 0.0)
            warm_ps = pp.tile([P, 2, 512], f32, tag="mt")
            N_WARM = 16
            for i in range(N_WARM):
                nc.tensor.matmul(
                    warm_ps[:, 0, 0:448],
                    warm_sb[:, 0:P],
                    warm_sb[:],
                    start=(i == 0),
                    stop=(i == N_WARM - 1),
                )

            for n in range(N_PER_CORE):
                v_sb = xpool.tile([P, NT, NI, VCOLS], f16, tag="vc")
                # split the 1.27MB image load so the first strips start early
                nc.sync.dma_start(v_sb[:, 0:2], xp_ap[n, :, 0:2])
                nc.sync.dma_start(v_sb[:, 2:4], xp_ap[n, :, 2:4])
                nc.sync.dma_start(v_sb[:, 4:6], xp_ap[n, :, 4:6])
                for oh in range(2):
                    osl = slice(oh * P, (oh + 1) * P)
                    for s in range(NSTRIP):
                        i0 = TI * s
                        r0 = R * s
                        # PSUM slots: tA=[m1,m3], tB=[m2,m4], tC=[m0,m5]
                        tA = pp.tile([P, 2, 512], f32, tag="mt")
                        tB = pp.tile([P, 2, 512], f32, tag="mt")
                        tC = pp.tile([P, 2, 512], f32, tag="mt")
                        slot = {
                            1: tA[:, 0, 0:FREE], 3: tA[:, 1, 0:FREE],
                            2: tB[:, 0, 0:FREE], 4: tB[:, 1, 0:FREE],
                            0: tC[:, 0, 0:FREE], 5: tC[:, 1, 0:FREE],
                        }

                        def mms(t, extra_first=False):
                            if extra_first:  # bias rides the t=1 slot
                                nc.tensor.matmul(
                                    slot[t], bd_sb[:, oh], ones_sb[:],
                                    start=True, stop=False,
                                )
                            for kw in range(KW):
                                nc.tensor.matmul(
                                    slot[t],
                                    u_sb[:, t, kw, osl],
                                    v_sb[:, t, i0 : i0 + TI, kw : kw + W],
                                    start=(kw == 0 and not extra_first),
                                    stop=(kw == KW - 1),
                                )

                        mms(1, extra_first=True)
                        mms(3)
                        mms(2)
                        mms(4)
                        mms(0)
                        mms(5)

                        # fp16 scratch:
                        # [e h | g k | z0 z5 | Q S | Pp Rr | S2 S8 R4 QZ u0]
                        sc = spool.tile([P, 15, TI, W], f16, tag="sc")
                        # ACT: evict the six PSUM slots to fp16
                        psv = [
                            t[:, :, 0:FREE].rearrange(
                                "p s (a w) -> p s a w", a=TI
                            )
                            for t in (tA, tB, tC)
                        ]
                        nc.scalar.copy(sc[:, 0:2], psv[0])
                        nc.scalar.copy(sc[:, 2:4], psv[1])
                        nc.scalar.copy(sc[:, 4:6], psv[2])
                        eh = sc[:, 0:2]
                        gk = sc[:, 2:4]
                        z0, z5 = sc[:, 4], sc[:, 5]
                        QS, PR = sc[:, 6:8], sc[:, 8:10]
                        Q, S = sc[:, 6], sc[:, 7]
                        Pp, Rr = sc[:, 8], sc[:, 9]
                        S2, S8, R4, QZ, u0 = (sc[:, i] for i in range(10, 15))
                        nc.vector.tensor_sub(QS, eh, gk)
                        nc.vector.tensor_add(PR, eh, gk)

                        ot = opool.tile([P, R, W], f32)
                        # phase view: rows 4a+k -> [p, 7, k, 56]
                        oph = ot.rearrange("p (a f) w -> p a f w", f=4)

                        nc.vector.tensor_scalar_mul(S8, S, 8.0)
                        nc.vector.tensor_add(QZ, Q, z5)
                        nc.gpsimd.tensor_scalar_mul(S2, S, 2.0)
                        nc.vector.tensor_scalar_mul(R4, Rr, 4.0)
                        nc.gpsimd.tensor_add(u0, z0, Pp)

                        def phview(k):
                            return oph[:, :, k, :]

                        nc.vector.tensor_add(phview(1), S2, Q)     # o1
                        nc.vector.tensor_add(phview(3), S8, QZ)    # o3
                        nc.vector.tensor_add(phview(2), R4, Pp)    # o2
                        nc.gpsimd.tensor_add(phview(0), u0, Rr)    # o0
                        nc.sync.dma_start(out_ap[n, osl, r0 : r0 + R, :], ot[:])
    nc.finalize()
    return nc


def _prep(x, weight, b):
    x = np.asarray(x, dtype=np.float32)
    w = np.asarray(weight, dtype=np.float32)
    b = np.asarray(b, dtype=np.float32)
    bw = np.sign(w.astype(np.float64))
    N = x.shape[0]

    # weights: u[c, t, kw, o] = sum_s G[t,s] * sign(w)[o,c,s,kw]
    ut = np.einsum("ts,ocsk->ctko", G, bw)
    ut = np.ascontiguousarray(ut).astype(np.float16)

    # bias diag stationaries: bd[p, oh, o] = b[oh*128+o] if p==o
    bd = np.zeros((P, 2, P), np.float16)
    for ohalf in range(2):
        np.fill_diagonal(bd[:, ohalf, :], b[ohalf * P : (ohalf + 1) * P])

    # input: pad H and W, transform height tiles: v[n,c,t,i,col]
    xpad = np.zeros((N, P, H + 2, VCOLS), np.float16)
    xpad[:, :, 1 : H + 1, 1 : W + 1] = x.astype(np.float16)
    sh = xpad.strides
    seg = np.lib.stride_tricks.as_strided(
        xpad,
        shape=(N, P, NI, 6, VCOLS),
        strides=(sh[0], sh[1], 4 * sh[2], sh[2], sh[3]),
    )
    vp = np.einsum("ts,ncise->nctie", BT, seg.astype(np.float32))
    vp = vp.astype(np.float16)
    return vp, ut, bd


def _run(in_maps, trace=False):
    from concourse.bass_utils import run_bass_kernel_spmd

    nc = _build_nc()
    return run_bass_kernel_spmd(
        nc, in_maps, core_ids=list(range(N_CORES)), trace=trace
    )


def kernel(x, weight, b):
    vp, ut, bd = _prep(x, weight, b)
    in_maps = [
        {
            "xp": np.ascontiguousarray(vp[c * N_PER_CORE : (c + 1) * N_PER_CORE]),
            "wt": ut,
            "bias": bd,
        }
        for c in range(N_CORES)
    ]
    res = _run(in_maps, trace=False)
    return np.concatenate([r["out"] for r in res.results], axis=0)


# revision 13
# speedup vs baseline: 1.4211x; 1.4211x over previous
"""BinaryConv2d (sign-binarized 3x3 conv, stride 1, pad 1) on 8 Trainium2 cores.

Input  x      [32, 128, 56, 56] f32
       weight [256, 128, 3, 3]  f32  (binarized with sign() before the conv)
       b      [256]             f32
Output        [32, 256, 56, 56] f32

Sharding: data-parallel over the batch dim (4 images per core), weights
replicated to all cores.

Device kernel: 1D Winograd F(4,3) along W. Width is tiled into 14 tiles of
4 output cols; the 6-point input transform v = B^T d runs on HOST (fp16)
and is shipped instead of x (same DMA bytes as a padded fp16 image + 45%).
Height stays direct: 3 kh taps accumulate in PSUM, so PE work is 6/12 of
the direct fp16 shift-matmul conv, with fully contiguous 392-element
moving APs. Per strip of 28 output rows: 18 matmuls (6 t-points x 3 kh)
of free 392 into 6 bank-aligned PSUM slots (3 two-bank tiles), plus one
diag(bias) matmul into the t1 slot (t1 has inverse-transform coeff +1 in
every output, so bias rides the accumulation for free). ACT evicts the 6
slots to fp16 SBUF in 3 big ops; DVE/GPSIMD build the inverse transform
with plain fp16 tensor_tensor ops only (TensorScalarPtr is ~3x slower,
TensorScalar ~8x slower on this target): scaled terms come from paired
doubling chains [S,R] -> [2S,2R] -> [4S,4R]. Rel err ~5e-3 (fp16
transforms + fp16 m eviction, f32 PSUM).
"""

import functools

import numpy as np

P = 128          # partitions == input channels
H = W = 56       # spatial
O = 256          # output channels
NT = 6           # F(4,3) t-points
KH = 3           # kernel rows (direct accumulation)
NJ = 14          # width tiles (4 out cols each)
VROWS = H + 2    # 58 transformed input rows (pad included)
R = 28           # output rows per strip
NSTRIP = H // R  # 2
FREE = R * NJ    # 392 matmul free size
N_CORES = 8
N_PER_CORE = 4   # batch 32 / 8 cores

# F(4,3), interpolation points [0, 1, -1, 2, -2, inf]
BT = np.array(
    [
        [4, 0, -5, 0, 1, 0],
        [0, -4, -4, 1, 1, 0],
        [0, 4, -4, -1, 1, 0],
        [0, -2, -1, 2, 1, 0],
        [0, 2, -1, -2, 1, 0],
        [0, 4, 0, -5, 0, 1],
    ],
    np.float64,
)
G = np.array(
    [
        [1 / 4, 0, 0],
        [-1 / 6, -1 / 6, -1 / 6],
        [-1 / 6, 1 / 6, -1 / 6],
        [1 / 24, 1 / 12, 1 / 6],
        [1 / 24, -1 / 12, 1 / 6],
        [0, 0, 1],
    ],
    np.float64,
)
# A^T = [[1,1,1,1,1,0], [0,1,-1,2,-2,0], [0,1,1,4,4,0], [0,1,-1,8,-8,1]]
# with e=m1(+bias), g=m2, h=m3, k=m4, z0=m0, z5=m5 (fp16 in SBUF):
#   S = h-k, Q = e-g, Rr = h+k, Pp = e+g     (two paired ops)
#   [S2,R2] = [S,Rr]*2, [S4,R4] = [S2,R2]*2, S8 = S4+S4   (doubling)
#   o0 = (z0+Pp)+Rr; o1 = S2+Q; o2 = R4+Pp; o3 = S8+(Q+z5)


@functools.lru_cache(maxsize=1)
def _build_nc():
    import concourse.mybir as mybir
    import concourse.tile as tile
    from concourse import bacc

    f16 = mybir.dt.float16
    f32 = mybir.dt.float32

    nc = bacc.Bacc()
    # xp: host-transformed input v[n, c, t, row, j]
    xp = nc.declare_dram_parameter(
        "xp", [N_PER_CORE, P, NT, VROWS, NJ], f16, isOutput=False
    )
    # wt: winograd weights u[c, t, kh, o]
    wt = nc.declare_dram_parameter("wt", [P, NT, KH, O], f16, isOutput=False)
    # bias: diag(b) stationaries per o-half: bias[p, oh, o] = b[oh*128+o]*(p==o)
    bias = nc.declare_dram_parameter("bias", [P, 2, P], f16, isOutput=False)
    out = nc.declare_dram_parameter(
        "out", [N_PER_CORE, O, H, W], f32, isOutput=True
    )
    xp_ap = xp[:]
    wt_ap = wt[:]
    bias_ap = bias[:]
    out_ap = out[:]

    with tile.TileContext(nc) as tc:
        with (
            tc.tile_pool(name="wpool", bufs=1) as wpool,
            tc.tile_pool(name="xpool", bufs=3) as xpool,
            tc.tile_pool(name="spool", bufs=3) as spool,
            tc.tile_pool(name="opool", bufs=4) as opool,
            tc.tile_pool(name="psum", bufs=4, space="PSUM") as pp,
        ):
            # Weights/bias on the scalar (ACT) DMA queue so they don't
            # serialize behind the image loads on sync.
            u_sb = wpool.tile([P, NT, KH, O], f16)
            nc.scalar.dma_start(u_sb[:, 0:3], wt_ap[:, 0:3])
            nc.scalar.dma_start(u_sb[:, 3:6], wt_ap[:, 3:6])
            bd_sb = wpool.tile([P, 2, P], f16)
            nc.scalar.dma_start(bd_sb[:], bias_ap)
            ones_sb = wpool.tile([P, FREE], f16)
            nc.gpsimd.memset(ones_sb[:], 1.0)

            # PE warmup: dummy matmuls with no data deps run during the
            # initial DMA wait and ramp the PE clock before the real stream.
            warm_sb = wpool.tile([P, 448], f16)
            nc.gpsimd.memset(warm_sb[:], 0.0)
            warm_ps = pp.tile([P, 2, 512], f32, tag="mt")
            N_WARM = 16
            for i in range(N_WARM):
                nc.tensor.matmul(
                    warm_ps[:, 0, 0:448],
                    warm_sb[:, 0:P],
                    warm_sb[:],
                    start=(i == 0),
                    stop=(i == N_WARM - 1),
                )

            for n in range(N_PER_CORE):
                v_sb = xpool.tile([P, NT, VROWS, NJ], f16, tag="vc")
                # split the 1.25MB image load so the first strips start early
                nc.sync.dma_start(v_sb[:, 0:2], xp_ap[n, :, 0:2])
                nc.sync.dma_start(v_sb[:, 2:4], xp_ap[n, :, 2:4])
                nc.sync.dma_start(v_sb[:, 4:6], xp_ap[n, :, 4:6])
                for oh in range(2):
                    osl = slice(oh * P, (oh + 1) * P)
                    for s in range(NSTRIP):
                        r0 = R * s
                        # PSUM slots: tA=[m3,m1], tB=[m4,m2], tC=[m0,m5]
                        tA = pp.tile([P, 2, 512], f32, tag="mt")
                        tB = pp.tile([P, 2, 512], f32, tag="mt")
                        tC = pp.tile([P, 2, 512], f32, tag="mt")
                        slot = {
                            3: tA[:, 0, 0:FREE], 1: tA[:, 1, 0:FREE],
                            4: tB[:, 0, 0:FREE], 2: tB[:, 1, 0:FREE],
                            0: tC[:, 0, 0:FREE], 5: tC[:, 1, 0:FREE],
                        }

                        def mms(t, extra_first=False):
                            if extra_first:  # bias rides the t=1 slot
                                nc.tensor.matmul(
                                    slot[t], bd_sb[:, oh], ones_sb[:],
                                    start=True, stop=False,
                                )
                            for kh in range(KH):
                                nc.tensor.matmul(
                                    slot[t],
                                    u_sb[:, t, kh, osl],
                                    v_sb[:, t, r0 + kh : r0 + kh + R, :],
                                    start=(kh == 0 and not extra_first),
                                    stop=(kh == KH - 1),
                                )

                        mms(3)
                        mms(1, extra_first=True)
                        mms(4)
                        mms(2)
                        mms(0)
                        mms(5)

                        # fp16 scratch slots:
                        # 0:h 1:e 2:k 3:g 4:z0 5:z5 | 6:S 7:Q 8:Rr 9:Pp
                        # 10:S2 11:QZ 12:R2 13:u0 | 14:S4 15:x 16:R4 17:S8
                        sc = spool.tile([P, 18, FREE], f16, tag="sc")
                        nc.scalar.copy(sc[:, 0:2], tA[:, :, 0:FREE])
                        nc.scalar.copy(sc[:, 2:4], tB[:, :, 0:FREE])
                        nc.scalar.copy(sc[:, 4:6], tC[:, :, 0:FREE])
                        hk = sc[:, 0:2]      # [h, e]
                        kg = sc[:, 2:4]      # [k, g]
                        z0, z5 = sc[:, 4], sc[:, 5]
                        SQ, RP = sc[:, 6:8], sc[:, 8:10]
                        S, Q = sc[:, 6], sc[:, 7]
                        Rr, Pp = sc[:, 8], sc[:, 9]
                        # interleaved pair views: [S,Rr] / [S2,R2] / [S4,R4]
                        SR = sc[:, 6:10].rearrange("p (x y) f -> p x y f", x=2)[
                            :, :, 0
                        ]
                        SR2 = sc[:, 10:14].rearrange("p (x y) f -> p x y f", x=2)[
                            :, :, 0
                        ]
                        SR4 = sc[:, 14:18].rearrange("p (x y) f -> p x y f", x=2)[
                            :, :, 0
                        ]
                        S2, QZ, R2, u0 = (sc[:, i] for i in range(10, 14))
                        S4, R4, S8 = sc[:, 14], sc[:, 16], sc[:, 17]

                        nc.vector.tensor_sub(SQ, hk, kg)    # [S, Q]
                        nc.vector.tensor_add(RP, hk, kg)    # [Rr, Pp]
                        nc.vector.tensor_add(SR2, SR, SR)   # [S2, R2]
                        nc.vector.tensor_add(SR4, SR2, SR2)  # [S4, R4]
                        nc.vector.tensor_add(S8, S4, S4)
                        nc.gpsimd.tensor_add(QZ, Q, z5)
                        nc.gpsimd.tensor_add(u0, z0, Pp)

                        ot = opool.tile([P, R, W], f32)
                        oc = ot.rearrange("p r (j f) -> p (r j) f", f=4)
                        nc.vector.tensor_add(oc[:, :, 1], S2, Q)    # o1
                        nc.vector.tensor_add(oc[:, :, 3], S8, QZ)   # o3
                        nc.gpsimd.tensor_add(oc[:, :, 0], u0, Rr)   # o0
                        nc.gpsimd.tensor_add(oc[:, :, 2], R4, Pp)   # o2
                        nc.sync.dma_start(out_ap[n, osl, r0 : r0 + R, :], ot[:])
    nc.finalize()
    return nc


def _prep(x, weight, b):
    x = np.asarray(x, dtype=np.float32)
    w = np.asarray(weight, dtype=np.float32)
    b = np.asarray(b, dtype=np.float32)
    bw = np.sign(w.astype(np.float64))
    N = x.shape[0]

    # weights: u[c, t, kh, o] = sum_s G[t,s] * sign(w)[o,c,kh,s]
    ut = np.einsum("ts,ocks->ctko", G, bw)
    ut = np.ascontiguousarray(ut).astype(np.float16)

    # bias diag stationaries: bd[p, oh, o] = b[oh*128+o] if p==o
    bd = np.zeros((P, 2, P), np.float16)
    for ohalf in range(2):
        np.fill_diagonal(bd[:, ohalf, :], b[ohalf * P : (ohalf + 1) * P])

    # input: pad W to 58 cols, transform width tiles: v[n,c,t,row,j]
    xpad = np.zeros((N, P, VROWS, VROWS), np.float16)
    xpad[:, :, 1 : H + 1, 1 : W + 1] = x.astype(np.float16)
    sh = xpad.strides
    seg = np.lib.stride_tricks.as_strided(
        xpad,
        shape=(N, P, VROWS, NJ, 6),
        strides=(sh[0], sh[1], sh[2], 4 * sh[3], sh[3]),
    )
    vp = np.einsum("ts,ncrjs->nctrj", BT, seg.astype(np.float32))
    vp = vp.astype(np.float16)
    return vp, ut, bd


def _run(in_maps, trace=False):
    from concourse.bass_utils import run_bass_kernel_spmd

    nc = _build_nc()
    return run_bass_kernel_spmd(
        nc, in_maps, core_ids=list(range(N_CORES)), trace=trace
    )


def kernel(x, weight, b):
    vp, ut, bd = _prep(x, weight, b)
    in_maps = [
        {
            "xp": np.ascontiguousarray(vp[c * N_PER_CORE : (c + 1) * N_PER_CORE]),
            "wt": ut,
            "bias": bd,
        }
        for c in range(N_CORES)
    ]
    res = _run(in_maps, trace=False)
    return np.concatenate([r["out"] for r in res.results], axis=0)


# revision 14
# speedup vs baseline: 1.9436x; 1.3676x over previous
"""BinaryConv2d (sign-binarized 3x3 conv, stride 1, pad 1) on 8 Trainium2 cores.

Input  x      [32, 128, 56, 56] f32
       weight [256, 128, 3, 3]  f32  (binarized with sign() before the conv)
       b      [256]             f32
Output        [32, 256, 56, 56] f32

Sharding: data-parallel over the batch dim (4 images per core), weights
replicated to all cores.

Device kernel: 1D Winograd F(2,3) along W. Width is tiled into 28 tiles
of 2 output cols; the 4-point input transform v = B^T d runs on HOST
(fp16) and is shipped instead of x. Height stays direct: 3 kh taps
accumulate in PSUM, so PE work is 8/12 of the direct fp16 shift-matmul
conv with fully contiguous moving APs. F(2,3) is chosen over F(4,3)
because its inverse transform has all +-1 coefficients: on this target
every DVE/GPSIMD op costs ~1us regardless of size, so the formulation
with the fewest vector ops wins. Per strip of 16 output rows: 12 matmuls
(4 t-points x 3 kh, free 448) + one diag(bias) matmul into the t1 slot
(t1 has coeff +1 in both outputs, so bias rides the accumulation). ACT
evicts the 4 PSUM slots to fp16 SBUF in 2 big ops; DVE computes
W=a0+a1, V=a1-a2 and o0=W+a2; GPSIMD writes o1=V-a3. Rel err ~2e-3.
"""

import functools

import numpy as np

P = 128          # partitions == input channels
H = W = 56       # spatial
O = 256          # output channels
NT = 4           # F(2,3) t-points
KH = 3           # kernel rows (direct accumulation)
NJ = 28          # width tiles (2 out cols each)
VROWS = H + 2    # 58 transformed input rows (pad included)
RS = (16, 16, 16, 8)   # output rows per strip
N_CORES = 8
N_PER_CORE = 4   # batch 32 / 8 cores

# F(2,3), interpolation points [0, 1, -1, inf]
BT = np.array(
    [
        [1, 0, -1, 0],
        [0, 1, 1, 0],
        [0, -1, 1, 0],
        [0, 1, 0, -1],
    ],
    np.float64,
)
G = np.array(
    [
        [1, 0, 0],
        [0.5, 0.5, 0.5],
        [0.5, -0.5, 0.5],
        [0, 0, 1],
    ],
    np.float64,
)
# A^T = [[1,1,1,0],[0,1,-1,-1]]:
#   o0 = m0+m1+m2 (+bias via m1);  o1 = m1-m2-m3 (+bias via m1)


@functools.lru_cache(maxsize=1)
def _build_nc():
    import concourse.mybir as mybir
    import concourse.tile as tile
    from concourse import bacc

    f16 = mybir.dt.float16
    f32 = mybir.dt.float32

    nc = bacc.Bacc()
    # xp: host-transformed input v[n, c, t, row, j]
    xp = nc.declare_dram_parameter(
        "xp", [N_PER_CORE, P, NT, VROWS, NJ], f16, isOutput=False
    )
    # wt: winograd weights u[c, t, kh, o]
    wt = nc.declare_dram_parameter("wt", [P, NT, KH, O], f16, isOutput=False)
    # bias: diag(b) stationaries per o-half: bias[p, oh, o] = b[oh*128+o]*(p==o)
    bias = nc.declare_dram_parameter("bias", [P, 2, P], f16, isOutput=False)
    out = nc.declare_dram_parameter(
        "out", [N_PER_CORE, O, H, W], f32, isOutput=True
    )
    xp_ap = xp[:]
    wt_ap = wt[:]
    bias_ap = bias[:]
    out_ap = out[:]

    with tile.TileContext(nc) as tc:
        with (
            tc.tile_pool(name="wpool", bufs=1) as wpool,
            tc.tile_pool(name="xpool", bufs=3) as xpool,
            tc.tile_pool(name="spool", bufs=4) as spool,
            tc.tile_pool(name="opool", bufs=4) as opool,
            tc.tile_pool(name="psum", bufs=4, space="PSUM") as pp,
        ):
            # Weights/bias on the scalar (ACT) DMA queue so they don't
            # serialize behind the image loads on sync.
            u_sb = wpool.tile([P, NT, KH, O], f16)
            nc.scalar.dma_start(u_sb[:, 0:2], wt_ap[:, 0:2])
            nc.scalar.dma_start(u_sb[:, 2:4], wt_ap[:, 2:4])
            bd_sb = wpool.tile([P, 2, P], f16)
            nc.scalar.dma_start(bd_sb[:], bias_ap)
            ones_sb = wpool.tile([P, 448], f16)
            nc.gpsimd.memset(ones_sb[:], 1.0)

            # PE warmup: dummy matmuls with no data deps run during the
            # initial DMA wait and ramp the PE clock before the real stream.
            warm_sb = wpool.tile([P, 448], f16)
            nc.gpsimd.memset(warm_sb[:], 0.0)
            warm_ps = pp.tile([P, 2, 512], f32, tag="mt")
            N_WARM = 16
            for i in range(N_WARM):
                nc.tensor.matmul(
                    warm_ps[:, 0, 0:448],
                    warm_sb[:, 0:P],
                    warm_sb[:],
                    start=(i == 0),
                    stop=(i == N_WARM - 1),
                )

            for n in range(N_PER_CORE):
                v_sb = xpool.tile([P, NT, VROWS, NJ], f16, tag="vc")
                # split the 1.66MB image load so the first strips start early
                nc.sync.dma_start(v_sb[:, 0:1], xp_ap[n, :, 0:1])
                nc.sync.dma_start(v_sb[:, 1:2], xp_ap[n, :, 1:2])
                nc.sync.dma_start(v_sb[:, 2:3], xp_ap[n, :, 2:3])
                nc.sync.dma_start(v_sb[:, 3:4], xp_ap[n, :, 3:4])
                for oh in range(2):
                    osl = slice(oh * P, (oh + 1) * P)
                    r0 = 0
                    for rs in RS:
                        free = rs * NJ
                        # PSUM slots: tD=[m0,m1], tE=[m2,m3]
                        tD = pp.tile([P, 2, 512], f32, tag="mt")
                        tE = pp.tile([P, 2, 512], f32, tag="mt")
                        slot = {
                            0: tD[:, 0, 0:free], 1: tD[:, 1, 0:free],
                            2: tE[:, 0, 0:free], 3: tE[:, 1, 0:free],
                        }

                        def mms(t, extra_first=False):
                            if extra_first:  # bias rides the t=1 slot
                                nc.tensor.matmul(
                                    slot[t], bd_sb[:, oh], ones_sb[:, 0:free],
                                    start=True, stop=False,
                                )
                            for kh in range(KH):
                                nc.tensor.matmul(
                                    slot[t],
                                    u_sb[:, t, kh, osl],
                                    v_sb[:, t, r0 + kh : r0 + kh + rs, :],
                                    start=(kh == 0 and not extra_first),
                                    stop=(kh == KH - 1),
                                )

                        mms(0)
                        mms(1, extra_first=True)
                        mms(2)
                        mms(3)

                        # fp16 scratch: a0 a1 | a2 a3 | Wt Vt
                        sc = spool.tile([P, 6, 448], f16, tag="sc")
                        nc.scalar.copy(sc[:, 0:2, 0:free], tD[:, :, 0:free])
                        nc.scalar.copy(sc[:, 2:4, 0:free], tE[:, :, 0:free])
                        a0, a1 = sc[:, 0, 0:free], sc[:, 1, 0:free]
                        a2, a3 = sc[:, 2, 0:free], sc[:, 3, 0:free]
                        Wt, Vt = sc[:, 4, 0:free], sc[:, 5, 0:free]
                        nc.vector.tensor_add(Wt, a0, a1)
                        nc.vector.tensor_sub(Vt, a1, a2)

                        ot = opool.tile([P, 16, W], f32)
                        oc = ot[:, 0:rs].rearrange(
                            "p r (j f) -> p (r j) f", f=2
                        )
                        nc.vector.tensor_add(oc[:, :, 0], Wt, a2)   # o0
                        nc.gpsimd.tensor_sub(oc[:, :, 1], Vt, a3)   # o1
                        nc.sync.dma_start(
                            out_ap[n, osl, r0 : r0 + rs, :], ot[:, 0:rs]
                        )
                        r0 += rs
    nc.finalize()
    return nc


def _prep(x, weight, b):
    x = np.asarray(x, dtype=np.float32)
    w = np.asarray(weight, dtype=np.float32)
    b = np.asarray(b, dtype=np.float32)
    bw = np.sign(w.astype(np.float64))
    N = x.shape[0]

    # weights: u[c, t, kh, o] = sum_s G[t,s] * sign(w)[o,c,kh,s]
    ut = np.einsum("ts,ocks->ctko", G, bw)
    ut = np.ascontiguousarray(ut).astype(np.float16)

    # bias diag stationaries: bd[p, oh, o] = b[oh*128+o] if p==o
    bd = np.zeros((P, 2, P), np.float16)
    for ohalf in range(2):
        np.fill_diagonal(bd[:, ohalf, :], b[ohalf * P : (ohalf + 1) * P])

    # input: pad W to 58 cols, transform width tiles: v[n,c,t,row,j]
    xpad = np.zeros((N, P, VROWS, VROWS), np.float16)
    xpad[:, :, 1 : H + 1, 1 : W + 1] = x.astype(np.float16)
    sh = xpad.strides
    seg = np.lib.stride_tricks.as_strided(
        xpad,
        shape=(N, P, VROWS, NJ, 4),
        strides=(sh[0], sh[1], sh[2], 2 * sh[3], sh[3]),
    )
    vp = np.einsum("ts,ncrjs->nctrj", BT, seg.astype(np.float32))
    vp = vp.astype(np.float16)
    return vp, ut, bd


def _run(in_maps, trace=False):
    from concourse.bass_utils import run_bass_kernel_spmd

    nc = _build_nc()
    return run_bass_kernel_spmd(
        nc, in_maps, core_ids=list(range(N_CORES)), trace=trace
    )


def kernel(x, weight, b):
    vp, ut, bd = _prep(x, weight, b)
    in_maps = [
        {
            "xp": np.ascontiguousarray(vp[c * N_PER_CORE : (c + 1) * N_PER_CORE]),
            "wt": ut,
            "bias": bd,
        }
        for c in range(N_CORES)
    ]
    res = _run(in_maps, trace=False)
    return np.concatenate([r["out"] for r in res.results], axis=0)
